# revision 18
# baseline (speedup 1.0000x reference)
"""Distributed Trainium2 Bass kernel for a 16-head causal RoPE attention layer.

Problem: B=2, T=2048, D=1024, H=16, HS=64 (fp32 reference).

Sharding (8 cores): core = b*4 + g, b in {0,1} (batch), g in {0..3} (group of
4 heads).  Each core computes Q/K/V projections for its 256 head-dims, runs
causal attention for its 4 heads, and applies its 256-row slice of Wo,
producing a partial [T, D] output.  The host sums the 4 partials per batch
and adds bo.  No on-device collectives.

v5: all-bf16 compute (fp8 q/k measured 1.7e-2 rel err from softmax weight
noise -- too close to the 2e-2 gate), with the time recovered by keeping
the PE dense (Trainium2's PE downclocks 2.4->1.2GHz after any pipeline
gap and needs ~3us of continuous work to ramp back):
  - attention Y matmuls are software-pipelined TWO key-tiles behind their
    scores, so Y(kt-2) never waits on exp(kt-2) (it finished during the
    scores of kt-1/kt) and the PE stream has no per-kt dependency stall.
  - small independent PE units (projection chunks, V tiles, outproj
    T-tiles) are scattered one-per-two-kt-steps inside the attention
    quarters as fillers, sized so the per-kt PE time tracks the ~1.2us
    exp op and the ACT stream is never starved of scores.
  - q/k biases ride the PSUM eviction as a DVE tensor_scalar add (bias is
    per-partition in the q^T layout) -- no rank-1 bias matmuls.
  - causal masking: the diagonal 128x128 strict upper triangle is added
    as a -4e5 stationary matmul into the scores psum (PE), so exp
    underflows to 0 and DVE never touches the exp->Y path.
  - V bias via rank-1 matmul; normalize via ln/exp on ACT (shared table).
ACT is the bottleneck (~90us exp + ~21us ln/exp normalize); PE (~105us)
runs just under it when dense, DVE ~85us, Pool ~23us (RoPE adds).
PSUM: scores 2x2 banks + Y 2 banks + proj/outproj ring 2x1 = 8 exactly.
"""

import numpy as np
import ml_dtypes

import concourse.bass as bass
import concourse.mybir as mybir
import concourse.tile as tile
from concourse.bass_utils import run_bass_kernel_spmd

BF16 = mybir.dt.bfloat16
F32 = mybir.dt.float32

B, T, D = 2, 2048, 1024
H, HS = 16, 64
THETA = 10000.0
NCORES = 8
HG = 4            # heads per core
HD = HG * HS      # head dims per core = 256
SCALE = 1.0 / 8.0  # 1/sqrt(HS)
NEG = -1.0e5       # additive mask for padded keys (exp underflows to 0)
NEGM = -4.0e5      # diagonal-mask matmul constant (survives bf16 p exactly as 0)

_NC = None


_SELF_SEM = {
    "EngineType.Activation": "Activation_",
    "EngineType.DVE": "DVE_",
    "EngineType.PE": "PE_",
    "EngineType.Pool": "Pool_",
}


def _split_multi_waits(nc):
    """walrus codegen accepts at most ONE semaphore wait per engine
    instruction (the 64B ISA structs have a single EVENTS slot); Tile's
    scheduler freely emits several.  Hoist all but the last wait of each
    instruction onto inserted same-engine EventSemaphore (poll_sem) ops,
    which preserves semantics exactly (engines execute sequentially).

    Additionally drop ge-waits on the instruction's OWN engine semaphore
    for compute engines: those guard WAW/WAR against earlier instructions
    of the same in-order engine, which program order already guarantees."""
    def _names(args):
        out = set()
        for a in args:
            for attr in ("memref", "name"):
                v = getattr(a, attr, None)
                if isinstance(v, str):
                    out.add(v.removesuffix("_set"))
            t = getattr(a, "tensor", None)
            if t is not None and isinstance(getattr(t, "name", None), str):
                out.add(t.name)
        return out

    eng_written = {}
    eng_read = {}
    _COMPUTE = {"InstActivation", "InstTensorTensor", "InstTensorCopy",
                "InstMatmult", "InstLdweights", "InstMemset",
                "InstTensorScalarPtr", "InstTensorReduce"}
    for f in nc.m.functions:
        for blk in f.blocks:
            for inst in blk.instructions:
                if type(inst).__name__ in _COMPUTE:
                    e = str(inst.engine)
                    eng_written.setdefault(e, set()).update(_names(inst.outs))
                    eng_read.setdefault(e, set()).update(_names(inst.ins))

    n = 0
    for f in nc.m.functions:
        for blk in f.blocks:
            il = blk.instructions
            i = 0
            while i < len(il):
                inst = il[i]
                si = inst.sync_info
                if si is None or not si.on_wait:
                    i += 1
                    continue
                waits = list(si.on_wait)
                eng = str(inst.engine)
                selfpfx = _SELF_SEM.get(eng)
                if (selfpfx is not None
                        and type(inst).__name__ in (
                            "InstActivation", "InstMatmult", "InstLdweights",
                            "InstTensorTensor", "InstTensorCopy", "InstMemset")
                        and not (_names(inst.ins) & eng_written.get(eng, set()))
                        and not (_names(inst.outs) & eng_read.get(eng, set()))):
                    kept = [w for w in waits
                            if not (w.wait_mode == "sem-ge-imm"
                                    and w.ant_name.startswith(selfpfx))]
                    if len(kept) != len(waits):
                        waits = kept
                        inst.sync_info = mybir.SyncInfo(
                            on_wait=waits, on_update=list(si.on_update))
                if len(waits) > 1:
                    for w in waits[:-1]:
                        es = mybir.InstEventSemaphore(name=f"I-wsplit-{n}")
                        n += 1
                        es.engine = inst.engine
                        es.sync_info = mybir.SyncInfo(on_wait=[w], on_update=[])
                        nc.register_instruction(es)
                        il.insert(i, es)
                        i += 1
                    inst.sync_info = mybir.SyncInfo(
                        on_wait=[waits[-1]], on_update=list(si.on_update))
                i += 1
    return n


def _dedup_ldweights(nc):
    """bass emits one InstLdweights per InstMatmult.  When a later
    InstLdweights loads the IDENTICAL weights AP that is already resident
    in the PE array (no other InstLdweights in between), the reload is
    redundant: MATMUL does not self-load for 16-bit dtypes.  Delete it,
    folding its waits into the following matmult."""
    def fp(inst):
        a = inst.ins[0]
        return (a.memref, a.offset, str(a.ap), str(a.dtype),
                str(getattr(inst, "perf_mode", None)))

    n = 0
    for f in nc.m.functions:
        for blk in f.blocks:
            il = blk.instructions
            last = None
            i = 0
            while i < len(il):
                inst = il[i]
                tn = type(inst).__name__
                if tn == "InstLdweights":
                    cur = fp(inst)
                    si = inst.sync_info
                    if cur == last and (si is None or not si.on_update):
                        waits = list(si.on_wait) if si is not None else []
                        if waits:
                            j = i + 1
                            while (j < len(il)
                                   and type(il[j]).__name__ != "InstMatmult"):
                                j += 1
                            if j == len(il):
                                i += 1
                                continue
                            mm = il[j]
                            msi = mm.sync_info
                            mw = list(msi.on_wait) if msi is not None else []
                            mu = list(msi.on_update) if msi is not None else []
                            mm.sync_info = mybir.SyncInfo(
                                on_wait=waits + mw, on_update=mu)
                        del il[i]
                        n += 1
                        continue
                    last = cur
                i += 1
    return n


def build_nc():
    nc = bass.Bass()

    xT = nc.declare_dram_parameter("xT", [D, T], BF16, isOutput=False)
    wq = nc.declare_dram_parameter("wq", [D, HD], BF16, isOutput=False)
    wk = nc.declare_dram_parameter("wk", [D, HD], BF16, isOutput=False)
    wv = nc.declare_dram_parameter("wv", [D, HD], BF16, isOutput=False)
    wo = nc.declare_dram_parameter("wo", [HD, D], BF16, isOutput=False)
    # per-partition bias columns: [:, 2*ti+c2] = bias for q/k (ti) pair c2
    bcol = nc.declare_dram_parameter("bcol", [128, 4], F32, isOutput=False)
    bvrow = nc.declare_dram_parameter("bvrow", [1, HD], BF16, isOutput=False)
    cos2 = nc.declare_dram_parameter("cos2", [128, T], BF16, isOutput=False)
    sin2 = nc.declare_dram_parameter("sin2", [128, T], BF16, isOutput=False)
    # [tri | tri]: upper-triangular multiplicative mask for the diag block
    tri2 = nc.declare_dram_parameter("tri2", [128, 256], BF16, isOutput=False)
    kb = nc.declare_dram_parameter("kb", [T], F32, isOutput=False)
    out = nc.declare_dram_parameter("out", [T, D], BF16, isOutput=True)

    NK = T // 128   # 16 key tiles

    with tile.TileContext(nc) as tc:
        with (
            tc.tile_pool(name="const", bufs=1) as cpool,
            tc.tile_pool(name="xw", bufs=1) as xwpool,
            tc.tile_pool(name="qk", bufs=1) as qkpool,
            tc.tile_pool(name="raw", bufs=3) as rawpool,
            tc.tile_pool(name="p", bufs=6) as ppool,
            tc.tile_pool(name="rec", bufs=2) as rpool,
            tc.tile_pool(name="yr", bufs=2) as yrpool,
            tc.tile_pool(name="ev", bufs=3) as evpool,
            tc.tile_pool(name="psP", bufs=2, space="PSUM") as psP,
            tc.tile_pool(name="psS", bufs=2, space="PSUM") as psS,
            tc.tile_pool(name="psY", bufs=1, space="PSUM") as psY,
        ):
            # ---- constant / weight loads ----
            # wq + the first xT column-chunks gate the first matmul groups;
            # xT is loaded in 512-col chunks so projections start early.
            wq_sb = xwpool.tile([128, 8, HD], BF16, tag="wq")
            wk_sb = xwpool.tile([128, 8, HD], BF16, tag="wk")
            wv_sb = xwpool.tile([128, 8, HD], BF16, tag="wv")
            wo_sb = xwpool.tile([128, 2, D], BF16, tag="wo")
            bcol_sb = cpool.tile([128, 4], F32, tag="bcol")
            bvrow_sb = cpool.tile([1, HD], BF16, tag="bvrow")
            ones_sb = cpool.tile([1, 128], BF16, tag="ones")
            wq_r = wq.ap().rearrange("(c p) n -> p c n", p=128)
            wk_r = wk.ap().rearrange("(c p) n -> p c n", p=128)
            # pair-0 (c2=0) weight halves first: the first attention quarter
            # needs only these
            nc.sync.dma_start(wq_sb[:, :, 0:128], wq_r[:, :, 0:128])
            nc.sync.dma_start(bcol_sb[:], bcol.ap())
            nc.sync.dma_start(bvrow_sb[:], bvrow.ap())
            nc.vector.memset(ones_sb[:], 1.0)

            xts = []
            for dc in range(8):
                xt = xwpool.tile([128, T], BF16, tag=f"xt{dc}", name=f"xt{dc}")
                xts.append(xt)

            def load_xt_tr(tr):
                for dc in range(8):
                    nc.sync.dma_start(
                        xts[dc][:, tr * 512:(tr + 1) * 512],
                        xT[dc * 128:(dc + 1) * 128, tr * 512:(tr + 1) * 512],
                    )

            cos_sb = cpool.tile([128, T], BF16, tag="cos")
            sin_sb = cpool.tile([128, T], BF16, tag="sin")
            tri2_sb = cpool.tile([128, 2, 128], BF16, tag="tri2")
            kb_sb = cpool.tile([128, NK], F32, tag="kb")
            load_xt_tr(0)
            nc.sync.dma_start(wk_sb[:, :, 0:128], wk_r[:, :, 0:128])
            nc.sync.dma_start(cos_sb[:, 0:512], cos2[:, 0:512])
            nc.sync.dma_start(sin_sb[:, 0:512], sin2[:, 0:512])
            nc.sync.dma_start(wv_sb[:], wv.ap().rearrange("(c p) n -> p c n", p=128))
            nc.sync.dma_start(
                tri2_sb[:], tri2.ap().rearrange("p (two q) -> p two q", two=2))
            nc.sync.dma_start(kb_sb[:], kb.ap().rearrange("(t p) -> p t", p=128))
            load_xt_tr(1)
            nc.sync.dma_start(cos_sb[:, 512:T], cos2[:, 512:T])
            nc.sync.dma_start(sin_sb[:, 512:T], sin2[:, 512:T])
            load_xt_tr(2)
            load_xt_tr(3)
            nc.sync.dma_start(wq_sb[:, :, 128:256], wq_r[:, :, 128:256])
            nc.sync.dma_start(wk_sb[:, :, 128:256], wk_r[:, :, 128:256])
            nc.sync.dma_start(wo_sb[:], wo.ap().rearrange("(c p) n -> p c n", p=128))

            # persistent [128, T] tiles: 2 heads each (rows 0:64 / 64:128)
            qT = [qkpool.tile([128, T], BF16, tag=f"qT{c}", name=f"qT{c}") for c in range(2)]
            kT = [qkpool.tile([128, T], BF16, tag=f"kT{c}", name=f"kT{c}") for c in range(2)]
            yT = [qkpool.tile([128, T], BF16, tag=f"yT{c}", name=f"yT{c}") for c in range(2)]

            # ---- Q^T / K^T projection + RoPE, one 512-col chunk ----
            # ti: 0 = q, 1 = k (selects bias column)
            def proj_qk_chunk(ti, wsb, c2, fin, tr):
                lo, hi = tr * 512, (tr + 1) * 512
                ps = psP.tile([128, 512], F32, tag="pp")
                for dc in range(8):
                    nc.tensor.matmul(
                        ps[:],
                        wsb[:, dc, c2 * 128:(c2 + 1) * 128],
                        xts[dc][:, lo:hi],
                        start=(dc == 0),
                        stop=(dc == 7),
                    )
                # eviction cast + bias add in one DVE pass (bias is
                # per-partition in the q^T layout)
                raw = rawpool.tile([128, 512], BF16, tag="raw")
                nc.vector.tensor_scalar_add(
                    raw[:], ps[:], bcol_sb[:, 2 * ti + c2:2 * ti + c2 + 1])
                # RoPE: fin = raw*cos + rot(raw)*sin_signed
                f = fin
                for (do, di) in ((0, 32), (32, 0), (64, 96), (96, 64)):
                    nc.vector.tensor_copy(f[do:do + 32, lo:hi], raw[di:di + 32, :])
                nc.vector.tensor_mul(f[:, lo:hi], f[:, lo:hi], sin_sb[:, lo:hi])
                nc.vector.tensor_mul(raw[:], raw[:], cos_sb[:, lo:hi])
                # final add on Pool (idle) to unload DVE
                nc.gpsimd.tensor_add(f[:, lo:hi], f[:, lo:hi], raw[:])

            # ---- V projection (bf16, bias as rank-1, ones denominator cols)
            vts = [None] * NK

            def proj_v(kt):
                ps = psP.tile([128, HD], F32, tag="pp")
                for dc in range(8):
                    nc.tensor.matmul(
                        ps[:],
                        xts[dc][:, kt * 128:(kt + 1) * 128],
                        wv_sb[:, dc, :],
                        start=(dc == 0),
                        stop=False,
                    )
                nc.tensor.matmul(
                    ps[:], ones_sb[0:1, :], bvrow_sb[0:1, :],
                    start=False, stop=True,
                )
                vt = xwpool.tile([128, HG, 128], BF16, tag=f"v{kt}", name=f"v{kt}")
                nc.vector.tensor_copy(
                    vt[:, :, 0:64],
                    ps[:].rearrange("p (h d) -> p h d", h=HG),
                )
                nc.vector.memset(vt[:, :, 64:128], 1.0)
                vts[kt] = vt

            # ---- attention: one flat driver over (pair, quarter, kt) ----
            # scores/probs/Y for both heads side by side in one [128, 2, 512]
            # tile.  Y matmuls run TWO key-tiles behind their scores (so they
            # never wait on exp), and the pipeline runs ACROSS quarter
            # boundaries: a quarter's trailing Y matmuls interleave with the
            # next quarter's scores.  After the stop-Y, the y psum is evicted
            # raw (f32) to SBUF on DVE, freeing the single psY buffer ~1.1us
            # later; the ln/exp normalize reads the SBUF copy and is emitted
            # under the NEXT quarter's exp stream so ACT never waits for it.
            # Fillers (independent PE units, 0.4-1us) are popped one per kt.
            pend = []     # [(y, c2, kt, c, p, start, stop, fin)]

            def emit_y():
                y, c2, kt, c, p, st, sp, fin = pend.pop(0)
                for h in (0, 1):
                    nc.tensor.matmul(
                        y[:, h, c:],
                        vts[kt][:, 2 * c2 + h, :],
                        p[:, h, c:],
                        start=st,
                        stop=sp,
                        skip_group_check=True,
                    )
                if sp:
                    fin()

            def attn_quarter(c2, qq, fillers=(), norm_prev=None, tail=False):
                fillers = list(fillers)
                qlo = qq * 512
                last = 4 * qq + 3
                y = psY.tile([128, 2, 512], F32, tag="y", name=f"y{c2}_{qq}")

                def fin():
                    # raw eviction (numerators + denominators) to SBUF f32;
                    # the normalize is emitted later, under the next
                    # quarter's exp stream.
                    yraw = yrpool.tile([128, 2, 512], F32, tag="yr")
                    nc.vector.tensor_copy(yraw[:], y[:])
                    norm_q.append((c2, qq, yraw))

                for kt in range(last + 1):
                    j = kt - 4 * qq
                    c = j * 128 if j >= 0 else 0   # first valid col (diag trim)
                    ksl = slice(kt * 128, (kt + 1) * 128)
                    qsl = slice(qlo + c, qlo + 512)
                    s = psS.tile([128, 2, 512], F32, tag="s")
                    # two row-group-concurrent 64-row score matmuls
                    nc.tensor.matmul(
                        s[:, 0, c:], kT[c2][0:64, ksl], qT[c2][0:64, qsl],
                        start=True, stop=True,
                    )
                    nc.tensor.matmul(
                        s[:, 1, c:], kT[c2][64:128, ksl], qT[c2][64:128, qsl],
                        start=True, stop=True,
                    )
                    p = ppool.tile([128, 2, 512], BF16, tag="p")
                    nc.scalar.activation(
                        p[:, :, c:], s[:, :, c:],
                        mybir.ActivationFunctionType.Exp,
                        bias=kb_sb[:, kt:kt + 1], scale=SCALE,
                    )
                    if j >= 0:
                        # diagonal block: multiplicative triangular mask on
                        # DVE (p is consumed by Y two kt steps later)
                        nc.vector.tensor_mul(
                            p[:, :, c:c + 128], p[:, :, c:c + 128], tri2_sb[:])
                    if kt == 2 and norm_prev is not None:
                        # the previous quarter's stop-Y (and its raw
                        # eviction) was emitted at kt1; its normalize rides
                        # here, under this quarter's exp stream
                        norm_prev()
                    pend.append((y, c2, kt, c, p, kt == 0, kt == last, fin))
                    if len(pend) > 2:
                        emit_y()
                    if fillers:
                        f = fillers.pop(0)
                        if f is not None:
                            f()
                if tail:
                    while pend:
                        emit_y()
                    emit_norm()

            norm_q = []   # completed quarters awaiting normalize

            def emit_norm():
                c2, qq, yraw = norm_q.pop(0)
                qlo = qq * 512
                # 1/r = exp(-ln r): ln and exp share an ACT table (no reload)
                lnr = rpool.tile([64, 2, 512], F32, tag="lnr")
                rec = rpool.tile([64, 2, 512], F32, tag="rec")
                nc.scalar.activation(
                    lnr[:], yraw[64:128, :, :],
                    mybir.ActivationFunctionType.Ln)
                nc.scalar.activation(
                    rec[:], lnr[:], mybir.ActivationFunctionType.Exp,
                    scale=-1.0)
                nc.vector.tensor_mul(
                    yT[c2][0:64, qlo:qlo + 512], yraw[0:64, 0, :], rec[:, 0, :])
                nc.vector.tensor_mul(
                    yT[c2][64:128, qlo:qlo + 512], yraw[0:64, 1, :], rec[:, 1, :])

            # outproj split in two halves (one per head pair), each using one
            # psum tile per wo column half; the yT stationary slice serves
            # both column halves back-to-back (dedup deletes the reload).
            op_state = {}

            def outproj_half(tt, c2, on_scalar=False):
                if c2 == 0:
                    op_state[tt] = [
                        psP.tile([128, 512], F32, tag="pp", name=f"po{t}")
                        for t in range(2)]
                pss = op_state[tt]
                for dr in range(2):
                    nc.tensor.matmul(
                        pss[dr][:],
                        yT[c2][:, tt * 128:(tt + 1) * 128],
                        wo_sb[:, c2, dr * 512:(dr + 1) * 512],
                        start=(c2 == 0),
                        stop=(c2 == 1),
                        skip_group_check=True,
                    )
                if c2 == 1:
                    for dr in range(2):
                        ev = evpool.tile([128, 512], BF16, tag="ev")
                        if on_scalar:
                            nc.scalar.activation(
                                ev[:], pss[dr][:],
                                mybir.ActivationFunctionType.Identity)
                        else:
                            nc.vector.tensor_copy(ev[:], pss[dr][:])
                        nc.sync.dma_start(
                            out[tt * 128:(tt + 1) * 128,
                                dr * 512:(dr + 1) * 512],
                            ev[:],
                        )

            # projection chunks split in two 4-matmul halves for finer
            # filler placement; the psum tile is shared via pr_state.
            pr_state = {}

            def proj_half(ti, wsb, c2, fin, tr, second):
                key = (ti, c2, tr)
                lo, hi = tr * 512, (tr + 1) * 512
                if not second:
                    pr_state[key] = psP.tile([128, 512], F32, tag="pp",
                                             name="prh")
                ps = pr_state[key]
                for dc in (range(4, 8) if second else range(4)):
                    nc.tensor.matmul(
                        ps[:],
                        wsb[:, dc, c2 * 128:(c2 + 1) * 128],
                        xts[dc][:, lo:hi],
                        start=(dc == 0),
                        stop=(dc == 7),
                    )
                if second:
                    proj_rope(ti, c2, fin, tr, ps)

            def proj_rope(ti, c2, fin, tr, ps):
                lo, hi = tr * 512, (tr + 1) * 512
                raw = rawpool.tile([128, 512], BF16, tag="raw")
                nc.vector.tensor_scalar_add(
                    raw[:], ps[:], bcol_sb[:, 2 * ti + c2:2 * ti + c2 + 1])
                f = fin
                for (do, di) in ((0, 32), (32, 0), (64, 96), (96, 64)):
                    nc.vector.tensor_copy(f[do:do + 32, lo:hi], raw[di:di + 32, :])
                nc.vector.tensor_mul(f[:, lo:hi], f[:, lo:hi], sin_sb[:, lo:hi])
                nc.vector.tensor_mul(raw[:], raw[:], cos_sb[:, lo:hi])
                nc.gpsimd.tensor_add(f[:, lo:hi], f[:, lo:hi], raw[:])

            def F(fn, *a, **k):
                return lambda: fn(*a, **k)

            def PJ(ti, wsb, c2, fin, tr):
                return [F(proj_half, ti, wsb, c2, fin, tr, False),
                        F(proj_half, ti, wsb, c2, fin, tr, True)]

            def OP(tt, on_scalar=False):
                return [F(outproj_half, tt, 0),
                        F(outproj_half, tt, 1, on_scalar)]

            # ---- emission order == scheduler priority ----
            # quarter qq of pair c2 needs q/k chunks tr<=qq (of pair c2) and
            # V tiles kt<=4qq+3; each filler unit's inputs are ready at
            # least two kt steps ahead of first use.
            proj_qk_chunk(0, wq_sb, 0, qT[0], 0)
            proj_qk_chunk(1, wk_sb, 0, kT[0], 0)
            for kt in range(0, 4):
                proj_v(kt)
            attn_quarter(0, 0,
                         PJ(0, wq_sb, 0, qT[0], 1) + PJ(1, wk_sb, 0, kT[0], 1))
            attn_quarter(0, 1,
                         [F(proj_v, 4), F(proj_v, 5)]
                         + PJ(0, wq_sb, 0, qT[0], 2)
                         + [F(proj_v, 6), F(proj_v, 7)]
                         + PJ(1, wk_sb, 0, kT[0], 2),
                         norm_prev=emit_norm)
            attn_quarter(0, 2,
                         [F(proj_v, 8), F(proj_v, 9)]
                         + PJ(0, wq_sb, 0, qT[0], 3)
                         + [F(proj_v, 10), F(proj_v, 11)]
                         + PJ(1, wk_sb, 0, kT[0], 3)
                         + [F(proj_v, 12), F(proj_v, 13)],
                         norm_prev=emit_norm)
            attn_quarter(0, 3,
                         [F(proj_v, 14), F(proj_v, 15)]
                         + PJ(0, wq_sb, 1, qT[1], 0)
                         + PJ(1, wk_sb, 1, kT[1], 0)
                         + PJ(0, wq_sb, 1, qT[1], 1)
                         + PJ(1, wk_sb, 1, kT[1], 1)
                         + PJ(0, wq_sb, 1, qT[1], 2),
                         norm_prev=emit_norm)
            attn_quarter(1, 0,
                         PJ(1, wk_sb, 1, kT[1], 2),
                         norm_prev=emit_norm)
            attn_quarter(1, 1,
                         PJ(0, wq_sb, 1, qT[1], 3)
                         + PJ(1, wk_sb, 1, kT[1], 3)
                         + OP(0) + OP(1),
                         norm_prev=emit_norm)
            attn_quarter(1, 2,
                         [None, None]
                         + OP(2) + OP(3) + OP(4) + OP(5) + OP(6),
                         norm_prev=emit_norm)
            attn_quarter(1, 3,
                         [None, None]
                         + OP(7) + OP(8) + OP(9) + OP(10) + OP(11),
                         norm_prev=emit_norm, tail=True)
            for tt in range(12, 16):
                for f in OP(tt, on_scalar=True):
                    f()
    nd = _dedup_ldweights(nc)
    _split_multi_waits(nc)
    assert nd > 0, f"expected ldweights dedup to fire, got {nd}"
    return nc


def _rope_tables():
    inv_freq = 1.0 / (THETA ** (np.arange(0, HS, 2, dtype=np.float64) / HS))  # [32]
    t = np.arange(T, dtype=np.float64)
    fr = t[:, None] * inv_freq[None, :]          # [T, 32]
    emb = np.concatenate([fr, fr], axis=1)       # [T, 64]
    cos = np.cos(emb).T.astype(np.float32)       # [64, T]
    sin = np.sin(emb).T.astype(np.float32)       # [64, T]
    sin_signed = sin.copy()
    sin_signed[0:32] = -sin_signed[0:32]
    cos2 = np.concatenate([cos, cos], axis=0)            # [128, T]
    sin2 = np.concatenate([sin_signed, sin_signed], 0)   # [128, T]
    return cos2.astype(ml_dtypes.bfloat16), sin2.astype(ml_dtypes.bfloat16)


def _in_maps(x, attention_mask, Wq, bqv, Wk, bkv, Wv, bvv, Wo):
    cos2, sin2 = _rope_tables()
    tri = np.triu(np.ones((128, 128), np.float32))
    tri2 = np.concatenate([tri, tri], axis=1).astype(ml_dtypes.bfloat16)
    bf = ml_dtypes.bfloat16
    xTs = [np.ascontiguousarray(x[b].T).astype(bf) for b in range(B)]
    kbs = [
        np.where(attention_mask[b] != 0, 0.0, NEG).astype(np.float32)
        for b in range(B)
    ]
    maps = []
    for core in range(NCORES):
        b, g = core // 4, core % 4
        sl = slice(g * HD, (g + 1) * HD)
        # bias columns [128, 4]: [:, 2*ti+c2]
        bcol = np.stack([
            bqv[sl][0:128], bqv[sl][128:256],
            bkv[sl][0:128], bkv[sl][128:256],
        ], axis=1).astype(np.float32)
        maps.append({
            "xT": xTs[b],
            "wq": np.ascontiguousarray(Wq[:, sl]).astype(bf),
            "wk": np.ascontiguousarray(Wk[:, sl]).astype(bf),
            "wv": np.ascontiguousarray(Wv[:, sl]).astype(bf),
            "wo": np.ascontiguousarray(Wo[sl, :]).astype(bf),
            "bcol": bcol,
            "bvrow": bvv[sl].astype(bf).reshape(1, HD),
            "cos2": cos2,
            "sin2": sin2,
            "tri2": tri2,
            "kb": kbs[b],
        })
    return maps


def _run(inputs, trace=False):
    global _NC
    if _NC is None:
        _NC = build_nc()
    maps = _in_maps(
        np.asarray(inputs["x"]), np.asarray(inputs["attention_mask"]),
        np.asarray(inputs["Wq"]), np.asarray(inputs["bq"]),
        np.asarray(inputs["Wk"]), np.asarray(inputs["bk"]),
        np.asarray(inputs["Wv"]), np.asarray(inputs["bv"]),
        np.asarray(inputs["Wo"]),
    )
    res = run_bass_kernel_spmd(_NC, maps, core_ids=list(range(NCORES)), trace=trace)
    bo = np.asarray(inputs["bo"], np.float32)
    outs = []
    for b in range(B):
        acc = np.zeros((T, D), np.float32)
        for g in range(4):
            acc += np.asarray(res.results[b * 4 + g]["out"], np.float32)
        outs.append(acc + bo[None, :])
    return np.stack(outs, axis=0), res


def kernel(**inputs):
    out, _ = _run(inputs, trace=False)
    return out
